# revision 6
# baseline (speedup 1.0000x reference)
"""Trainium2 Bass kernel for nn_CustomConv2D (degenerate conv: only the last
input channel contributes; 3x3 VALID conv -> 64 out channels + bias).

Strategy (v4):
  - Tolerance is rel_err < 2e-2; fp16 end-to-end gives ~2e-4, so the im2col
    moving data AND the output are fp16 (in 1.81 MB, out 12.85 MB per core;
    host upcasts the result).
  - Host: slice x_padded[:, -1], build the 9-row im2col per batch WITHOUT
    zero padding rows ([pairs, seg, 18, 3136]), shard batch across 8 cores.
  - Input: 16 DMAs (one per pair x segment) on the GpSimd/SWDGE queue so
    the Scalar sequencer stays free for evacuations; rows land directly at
    partition offsets 32s..32s+17.
  - Inner loop: for each t (7 x 448 pixels per segment) the 4 concurrent
    quadrant matmuls (stationary replicated at partition 0/32/64/96,
    block-diagonal over the batch pair) write 4 DISTINCT BANKS of one
    [128, 2048] PSUM tile. A single strided-AP op evacuates all four banks
    with fused bias add, alternating Scalar/Vector engines (7 grouped
    evacs per pair instead of 28 small ones; per-op overhead ~400 ns).
  - Evacs t=0..3 write stageA, t=4..6 write stageB (seg-major layout), so
    each stage drains as ONE large contiguous DMA (1.84 / 1.38 MB) with no
    AP overlap with later evacs -> only 8 output DMAs per core, all big.
"""

import sys

if "/opt/trn_rl_repo" not in sys.path:
    sys.path.insert(0, "/opt/trn_rl_repo")

import numpy as np

B, CIN, COUT, KS = 64, 64, 64, 3
H, W, HP, WP = 112, 112, 114, 114
NPIX = H * W          # 12544
IMG = HP * WP         # 12996
NCORES = 8
BL = B // NCORES      # 8 local batches per core
PAIRS = BL // 2       # 4
KDIM = 2 * KS * KS    # 18
NSEG = 4              # pixel segments per pair (partition offsets 0/32/64/96)
SEGW = NPIX // NSEG   # 3136
NT = 448              # pixels per matmul; 7 * 448 == 3136
TPS = SEGW // NT      # 7 matmul tiles per segment
BANK = 512            # f32 elems per PSUM bank
TA = 4                # t-chunks staged in stageA (drained early)
TB = TPS - TA         # 3 t-chunks in stageB
WA, WB = TA * NT, TB * NT   # 1792, 1344

_CACHE = {}


def _build_bass():
    import bass_rust
    import concourse.bass as bass
    import concourse.bacc as bacc
    import concourse.mybir as mybir
    from concourse.tile import TileContext

    f32 = mybir.dt.float32
    f16 = mybir.dt.float16

    def apx(base_ap, extra_off, dims):
        """Custom access pattern on a tile: dims = [[stride, size], ...]
        (elements), first dim must be the partition dim."""
        return bass_rust.AP(base_ap.tensor, base_ap.offset + extra_off, dims)

    nc = bacc.Bacc("TRN2", target_bir_lowering=False, debug=False)
    mv = nc.declare_dram_parameter("mv", [PAIRS, NSEG, KDIM, SEGW], f16,
                                   isOutput=False)
    w2 = nc.declare_dram_parameter("w2", [128, 128], f16, isOutput=False)
    b2 = nc.declare_dram_parameter("b2", [128, 1], f32, isOutput=False)
    out = nc.declare_dram_parameter("out", [BL * COUT, NPIX], f16,
                                    isOutput=True)

    with TileContext(nc) as tc:
        with (
            tc.tile_pool(name="consts", bufs=1) as consts,
            tc.tile_pool(name="movp", bufs=PAIRS) as movp,
            tc.tile_pool(name="stageap", bufs=2) as stageap,
            tc.tile_pool(name="stagebp", bufs=2) as stagebp,
            tc.tile_pool(name="psump", bufs=2, space="PSUM") as psump,
        ):
            w2_t = consts.tile([128, 128], f16)
            nc.scalar.dma_start(out=w2_t[:], in_=w2[:])
            b2_t = consts.tile([128, 1], f32)
            nc.sync.dma_start(out=b2_t[:], in_=b2[:])

            # Prefetch every pair's moving tile. Pair 0's segments go on the
            # Scalar HWDGE queue (idle until the first evacuation, lower
            # first-byte latency); the rest on the SWDGE queue (idle Pool
            # sequencer).
            movs = []
            for pair in range(PAIRS):
                mov = movp.tile([128, SEGW], f16, tag="mov",
                                name=f"mov_{pair}")
                for s4 in range(NSEG):
                    eng = nc.scalar if pair == 0 else nc.gpsimd
                    eng.dma_start(
                        out=mov[32 * s4:32 * s4 + KDIM, :],
                        in_=mv[pair, s4])
                movs.append(mov)

            # t-chunks grouped into one stage tile + one drain DMA per
            # group: pair 0 drains its first chunks ASAP (prime the output
            # stream), pair 3 drains its last chunks individually (short
            # tail); middle pairs use big 2/2/3 groups.
            GROUPS = {
                0: [[0], [1], [2, 3], [4, 5, 6]],
                1: [[0, 1], [2, 3], [4, 5, 6]],
                2: [[0, 1], [2, 3], [4, 5, 6]],
                3: [[0, 1], [2, 3], [4], [5], [6]],
            }
            for pair in range(PAIRS):
                mov = movs[pair]
                groups = GROUPS[pair]
                gtiles = {}
                for gi, grp in enumerate(groups):
                    gw = NT * len(grp)
                    pool = stageap if gi % 2 == 0 else stagebp
                    gtiles[gi] = pool.tile([128, NSEG * gw], f16, tag="st",
                                           name=f"st_{pair}_{gi}")
                for gi, grp in enumerate(groups):
                    gw = NT * len(grp)
                    st = gtiles[gi]
                    for t in grp:
                        n0 = t * NT
                        pt = psump.tile([128, 4 * BANK], f32, tag="pt")
                        for seg in range(NSEG):
                            p0 = 32 * seg
                            nc.tensor.matmul(
                                pt[:, BANK * seg:BANK * seg + NT],
                                w2_t[p0:p0 + KDIM, :],
                                mov[p0:p0 + KDIM, n0:n0 + NT],
                                start=True, stop=True,
                                tile_position=(p0, 0))
                        # Grouped PSUM->SBUF evacuation of all 4 banks with
                        # fused bias add. Per-pair engine pattern A,D,A,D,...
                        # gives ACT (the faster engine) 4 of every 7.
                        in_ap = apx(pt[:], 0, [[4 * BANK, 128], [BANK, NSEG],
                                               [1, NT]])
                        out_ap = apx(st[:], NT * (t - grp[0]),
                                     [[NSEG * gw, 128], [gw, NSEG], [1, NT]])
                        if t % 2 == 0:
                            nc.scalar.activation(
                                out_ap, in_ap,
                                mybir.ActivationFunctionType.Identity,
                                bias=b2_t[:, :])
                        else:
                            nc.vector.tensor_scalar_add(out_ap, in_ap,
                                                        b2_t[:, :])
                    # Drain this group: stage cols gw*seg+j -> out cols
                    # SEGW*seg + NT*grp[0] + j.
                    nc.sync.dma_start(
                        out=apx(out[:], pair * 128 * NPIX + NT * grp[0],
                                [[NPIX, 128], [SEGW, NSEG], [1, gw]]),
                        in_=st[:])
    nc.compile()
    return nc


def _get_nc():
    if "nc" not in _CACHE:
        _CACHE["nc"] = _build_bass()
    return _CACHE["nc"]


def _prep_inputs(x_padded, weight, bias):
    x = np.asarray(x_padded, dtype=np.float32)
    wt = np.asarray(weight, dtype=np.float32)
    bs = np.asarray(bias, dtype=np.float32)

    xs3 = x[:, -1, :, :]                              # [64, 114, 114]
    win = np.lib.stride_tricks.sliding_window_view(xs3, (KS, KS), axis=(1, 2))
    # [64, 112, 112, 3, 3] -> [64, 9, 12544] with row k = (i, j) shift
    mov_all = win.transpose(0, 3, 4, 1, 2).reshape(B, KS * KS, NPIX)
    # -> [cores, pairs, 2, 9, NSEG, SEGW] -> [cores, pairs, NSEG, 18, SEGW]
    mov_r = mov_all.reshape(NCORES, PAIRS, 2, KS * KS, NSEG, SEGW)
    mov_h = np.ascontiguousarray(
        mov_r.transpose(0, 1, 4, 2, 3, 5).reshape(
            NCORES, PAIRS, NSEG, KDIM, SEGW)).astype(np.float16)

    wl = np.ascontiguousarray(wt[:, -1, :, :]).reshape(COUT, KS * KS)
    w2 = np.zeros((128, 128), np.float16)
    for s in range(NSEG):
        w2[32 * s: 32 * s + 9, 0:64] = wl.T
        w2[32 * s + 9: 32 * s + 18, 64:128] = wl.T
    b2 = np.tile(bs, 2).reshape(128, 1).astype(np.float32)
    return mov_h, w2, b2


def kernel(x_padded, weight, bias, in_height=112, in_width=112, **_unused):
    from concourse.bass_utils import run_bass_kernel_spmd

    mov_h, w2, b2 = _prep_inputs(x_padded, weight, bias)
    nc = _get_nc()
    in_maps = [
        {"mv": mov_h[c], "w2": w2, "b2": b2}
        for c in range(NCORES)
    ]
    res = run_bass_kernel_spmd(nc, in_maps, core_ids=list(range(NCORES)))
    outs = [
        np.asarray(res.results[c]["out"]).astype(np.float32).reshape(
            BL, COUT, H, W)
        for c in range(NCORES)
    ]
    return np.concatenate(outs, axis=0)


# revision 7
# speedup vs baseline: 1.0561x; 1.0561x over previous
"""Trainium2 Bass kernel for nn_CustomConv2D (degenerate conv: only the last
input channel contributes; 3x3 VALID conv -> 64 out channels + bias).

Strategy (v4):
  - Tolerance is rel_err < 2e-2; fp16 end-to-end gives ~2e-4, so the im2col
    moving data AND the output are fp16 (in 1.81 MB, out 12.85 MB per core;
    host upcasts the result).
  - Host: slice x_padded[:, -1], build the 9-row im2col per batch WITHOUT
    zero padding rows ([pairs, seg, 18, 3136]), shard batch across 8 cores.
  - Input: 16 DMAs (one per pair x segment) on the GpSimd/SWDGE queue so
    the Scalar sequencer stays free for evacuations; rows land directly at
    partition offsets 32s..32s+17.
  - Inner loop: for each t (7 x 448 pixels per segment) the 4 concurrent
    quadrant matmuls (stationary replicated at partition 0/32/64/96,
    block-diagonal over the batch pair) write 4 DISTINCT BANKS of one
    [128, 2048] PSUM tile. A single strided-AP op evacuates all four banks
    with fused bias add, alternating Scalar/Vector engines (7 grouped
    evacs per pair instead of 28 small ones; per-op overhead ~400 ns).
  - Evacs t=0..3 write stageA, t=4..6 write stageB (seg-major layout), so
    each stage drains as ONE large contiguous DMA (1.84 / 1.38 MB) with no
    AP overlap with later evacs -> only 8 output DMAs per core, all big.
"""

import sys

if "/opt/trn_rl_repo" not in sys.path:
    sys.path.insert(0, "/opt/trn_rl_repo")

import numpy as np

B, CIN, COUT, KS = 64, 64, 64, 3
H, W, HP, WP = 112, 112, 114, 114
NPIX = H * W          # 12544
IMG = HP * WP         # 12996
NCORES = 8
BL = B // NCORES      # 8 local batches per core
PAIRS = BL // 2       # 4
KDIM = 2 * KS * KS    # 18
NSEG = 4              # pixel segments per pair (partition offsets 0/32/64/96)
SEGW = NPIX // NSEG   # 3136
NT = 448              # pixels per matmul; 7 * 448 == 3136
TPS = SEGW // NT      # 7 matmul tiles per segment
BANK = 512            # f32 elems per PSUM bank
TA = 4                # t-chunks staged in stageA (drained early)
TB = TPS - TA         # 3 t-chunks in stageB
WA, WB = TA * NT, TB * NT   # 1792, 1344

_CACHE = {}


def _build_bass():
    import bass_rust
    import concourse.bass as bass
    import concourse.bacc as bacc
    import concourse.mybir as mybir
    from concourse.tile import TileContext

    f32 = mybir.dt.float32
    f16 = mybir.dt.float16

    def apx(base_ap, extra_off, dims):
        """Custom access pattern on a tile: dims = [[stride, size], ...]
        (elements), first dim must be the partition dim."""
        return bass_rust.AP(base_ap.tensor, base_ap.offset + extra_off, dims)

    nc = bacc.Bacc("TRN2", target_bir_lowering=False, debug=False)
    mv = nc.declare_dram_parameter("mv", [PAIRS, NSEG, KDIM, SEGW], f16,
                                   isOutput=False)
    w2 = nc.declare_dram_parameter("w2", [128, 128], f16, isOutput=False)
    b2 = nc.declare_dram_parameter("b2", [128, 1], f32, isOutput=False)
    out = nc.declare_dram_parameter("out", [BL * COUT, NPIX], f16,
                                    isOutput=True)

    with TileContext(nc) as tc:
        with (
            tc.tile_pool(name="consts", bufs=1) as consts,
            tc.tile_pool(name="movp", bufs=PAIRS) as movp,
            tc.tile_pool(name="stageap", bufs=2) as stageap,
            tc.tile_pool(name="stagebp", bufs=2) as stagebp,
            tc.tile_pool(name="psump", bufs=2, space="PSUM") as psump,
        ):
            w2_t = consts.tile([128, 128], f16)
            nc.scalar.dma_start(out=w2_t[:], in_=w2[:])
            b2_t = consts.tile([128, 1], f32)
            nc.sync.dma_start(out=b2_t[:], in_=b2[:])

            # Prefetch every pair's moving tile, ALL on the single SWDGE
            # queue (idle Pool sequencer): one queue = FIFO order, so pair
            # 0's tiles complete first. Pair 0's segments are split into a
            # small "starter" chunk (t=0 columns) + remainder so its first
            # matmuls can begin ~1.5 us earlier.
            movs = []
            for pair in range(PAIRS):
                mov = movp.tile([128, SEGW], f16, tag="mov",
                                name=f"mov_{pair}")
                for s4 in range(NSEG):
                    if pair == 0:
                        nc.gpsimd.dma_start(
                            out=mov[32 * s4:32 * s4 + KDIM, 0:NT],
                            in_=mv[pair, s4, :, 0:NT])
                        nc.gpsimd.dma_start(
                            out=mov[32 * s4:32 * s4 + KDIM, NT:SEGW],
                            in_=mv[pair, s4, :, NT:SEGW])
                    else:
                        nc.gpsimd.dma_start(
                            out=mov[32 * s4:32 * s4 + KDIM, :],
                            in_=mv[pair, s4])
                movs.append(mov)

            # t-chunks grouped into one stage tile + one drain DMA per
            # group: pair 0 drains its first chunks ASAP (prime the output
            # stream), pair 3 drains its last chunks individually (short
            # tail); middle pairs use big 2/2/3 groups.
            GROUPS = {
                0: [[0], [1], [2, 3], [4, 5, 6]],
                1: [[0, 1], [2, 3], [4, 5, 6]],
                2: [[0, 1], [2, 3], [4, 5, 6]],
                3: [[0, 1], [2, 3], [4], [5], [6]],
            }
            for pair in range(PAIRS):
                mov = movs[pair]
                groups = GROUPS[pair]
                gtiles = {}
                for gi, grp in enumerate(groups):
                    gw = NT * len(grp)
                    pool = stageap if gi % 2 == 0 else stagebp
                    gtiles[gi] = pool.tile([128, NSEG * gw], f16, tag="st",
                                           name=f"st_{pair}_{gi}")
                for gi, grp in enumerate(groups):
                    gw = NT * len(grp)
                    st = gtiles[gi]
                    for t in grp:
                        n0 = t * NT
                        pt = psump.tile([128, 4 * BANK], f32, tag="pt")
                        for seg in range(NSEG):
                            p0 = 32 * seg
                            nc.tensor.matmul(
                                pt[:, BANK * seg:BANK * seg + NT],
                                w2_t[p0:p0 + KDIM, :],
                                mov[p0:p0 + KDIM, n0:n0 + NT],
                                start=True, stop=True,
                                tile_position=(p0, 0))
                        # Grouped PSUM->SBUF evacuation of all 4 banks with
                        # fused bias add. Per-pair engine pattern A,D,A,D,...
                        # gives ACT (the faster engine) 4 of every 7.
                        in_ap = apx(pt[:], 0, [[4 * BANK, 128], [BANK, NSEG],
                                               [1, NT]])
                        out_ap = apx(st[:], NT * (t - grp[0]),
                                     [[NSEG * gw, 128], [gw, NSEG], [1, NT]])
                        if t % 2 == 0:
                            nc.scalar.activation(
                                out_ap, in_ap,
                                mybir.ActivationFunctionType.Identity,
                                bias=b2_t[:, :])
                        else:
                            nc.vector.tensor_scalar_add(out_ap, in_ap,
                                                        b2_t[:, :])
                    # Drain this group: stage cols gw*seg+j -> out cols
                    # SEGW*seg + NT*grp[0] + j.
                    nc.sync.dma_start(
                        out=apx(out[:], pair * 128 * NPIX + NT * grp[0],
                                [[NPIX, 128], [SEGW, NSEG], [1, gw]]),
                        in_=st[:])
    nc.compile()
    return nc


def _get_nc():
    if "nc" not in _CACHE:
        _CACHE["nc"] = _build_bass()
    return _CACHE["nc"]


def _prep_inputs(x_padded, weight, bias):
    x = np.asarray(x_padded, dtype=np.float32)
    wt = np.asarray(weight, dtype=np.float32)
    bs = np.asarray(bias, dtype=np.float32)

    xs3 = x[:, -1, :, :]                              # [64, 114, 114]
    win = np.lib.stride_tricks.sliding_window_view(xs3, (KS, KS), axis=(1, 2))
    # [64, 112, 112, 3, 3] -> [64, 9, 12544] with row k = (i, j) shift
    mov_all = win.transpose(0, 3, 4, 1, 2).reshape(B, KS * KS, NPIX)
    # -> [cores, pairs, 2, 9, NSEG, SEGW] -> [cores, pairs, NSEG, 18, SEGW]
    mov_r = mov_all.reshape(NCORES, PAIRS, 2, KS * KS, NSEG, SEGW)
    mov_h = np.ascontiguousarray(
        mov_r.transpose(0, 1, 4, 2, 3, 5).reshape(
            NCORES, PAIRS, NSEG, KDIM, SEGW)).astype(np.float16)

    wl = np.ascontiguousarray(wt[:, -1, :, :]).reshape(COUT, KS * KS)
    w2 = np.zeros((128, 128), np.float16)
    for s in range(NSEG):
        w2[32 * s: 32 * s + 9, 0:64] = wl.T
        w2[32 * s + 9: 32 * s + 18, 64:128] = wl.T
    b2 = np.tile(bs, 2).reshape(128, 1).astype(np.float32)
    return mov_h, w2, b2


def kernel(x_padded, weight, bias, in_height=112, in_width=112, **_unused):
    from concourse.bass_utils import run_bass_kernel_spmd

    mov_h, w2, b2 = _prep_inputs(x_padded, weight, bias)
    nc = _get_nc()
    in_maps = [
        {"mv": mov_h[c], "w2": w2, "b2": b2}
        for c in range(NCORES)
    ]
    res = run_bass_kernel_spmd(nc, in_maps, core_ids=list(range(NCORES)))
    outs = [
        np.asarray(res.results[c]["out"]).astype(np.float32).reshape(
            BL, COUT, H, W)
        for c in range(NCORES)
    ]
    return np.concatenate(outs, axis=0)


# revision 8
# speedup vs baseline: 1.0699x; 1.0131x over previous
"""Trainium2 Bass kernel for nn_CustomConv2D (degenerate conv: only the last
input channel contributes; 3x3 VALID conv -> 64 out channels + bias).

Strategy (v4):
  - Tolerance is rel_err < 2e-2; fp16 end-to-end gives ~2e-4, so the im2col
    moving data AND the output are fp16 (in 1.81 MB, out 12.85 MB per core;
    host upcasts the result).
  - Host: slice x_padded[:, -1], build the 9-row im2col per batch WITHOUT
    zero padding rows ([pairs, seg, 18, 3136]), shard batch across 8 cores.
  - Input: 16 DMAs (one per pair x segment) on the GpSimd/SWDGE queue so
    the Scalar sequencer stays free for evacuations; rows land directly at
    partition offsets 32s..32s+17.
  - Inner loop: for each t (7 x 448 pixels per segment) the 4 concurrent
    quadrant matmuls (stationary replicated at partition 0/32/64/96,
    block-diagonal over the batch pair) write 4 DISTINCT BANKS of one
    [128, 2048] PSUM tile. A single strided-AP op evacuates all four banks
    with fused bias add, alternating Scalar/Vector engines (7 grouped
    evacs per pair instead of 28 small ones; per-op overhead ~400 ns).
  - Evacs t=0..3 write stageA, t=4..6 write stageB (seg-major layout), so
    each stage drains as ONE large contiguous DMA (1.84 / 1.38 MB) with no
    AP overlap with later evacs -> only 8 output DMAs per core, all big.
"""

import sys

if "/opt/trn_rl_repo" not in sys.path:
    sys.path.insert(0, "/opt/trn_rl_repo")

import numpy as np

B, CIN, COUT, KS = 64, 64, 64, 3
H, W, HP, WP = 112, 112, 114, 114
NPIX = H * W          # 12544
IMG = HP * WP         # 12996
NCORES = 8
BL = B // NCORES      # 8 local batches per core
PAIRS = BL // 2       # 4
KDIM = 2 * KS * KS    # 18
NSEG = 4              # pixel segments per pair (partition offsets 0/32/64/96)
SEGW = NPIX // NSEG   # 3136
NT = 448              # pixels per matmul; 7 * 448 == 3136
TPS = SEGW // NT      # 7 matmul tiles per segment
BANK = 512            # f32 elems per PSUM bank
TA = 4                # t-chunks staged in stageA (drained early)
TB = TPS - TA         # 3 t-chunks in stageB
WA, WB = TA * NT, TB * NT   # 1792, 1344

_CACHE = {}


def _build_bass():
    import bass_rust
    import concourse.bass as bass
    import concourse.bacc as bacc
    import concourse.mybir as mybir
    from concourse.tile import TileContext

    f32 = mybir.dt.float32
    f16 = mybir.dt.float16

    def apx(base_ap, extra_off, dims):
        """Custom access pattern on a tile: dims = [[stride, size], ...]
        (elements), first dim must be the partition dim."""
        return bass_rust.AP(base_ap.tensor, base_ap.offset + extra_off, dims)

    nc = bacc.Bacc("TRN2", target_bir_lowering=False, debug=False)
    mv = nc.declare_dram_parameter("mv", [PAIRS, NSEG, KDIM, SEGW], f16,
                                   isOutput=False)
    w2 = nc.declare_dram_parameter("w2", [128, 128], f16, isOutput=False)
    b2 = nc.declare_dram_parameter("b2", [128, 1], f32, isOutput=False)
    out = nc.declare_dram_parameter("out", [BL * COUT, NPIX], f16,
                                    isOutput=True)

    with TileContext(nc) as tc:
        with (
            tc.tile_pool(name="consts", bufs=1) as consts,
            tc.tile_pool(name="movp", bufs=PAIRS) as movp,
            tc.tile_pool(name="sg0p", bufs=2) as sg0p,
            tc.tile_pool(name="sg1p", bufs=2) as sg1p,
            tc.tile_pool(name="sg2p", bufs=2) as sg2p,
            tc.tile_pool(name="stailp", bufs=3) as stailp,
            tc.tile_pool(name="psump", bufs=2, space="PSUM") as psump,
        ):
            w2_t = consts.tile([128, 128], f16)
            nc.scalar.dma_start(out=w2_t[:], in_=w2[:])
            b2_t = consts.tile([128, 1], f32)
            nc.sync.dma_start(out=b2_t[:], in_=b2[:])

            # Prefetch every pair's moving tile, ALL on the single SWDGE
            # queue (idle Pool sequencer): one queue = FIFO order, so pair
            # 0's tiles complete first. Pair 0 gets 4 small "starter"
            # chunks (t=0/1 columns) AHEAD of everything so its first
            # matmuls begin ASAP.
            movs = [movp.tile([128, SEGW], f16, tag="mov", name=f"mov_{p}")
                    for p in range(PAIRS)]
            for s4 in range(NSEG):
                nc.gpsimd.dma_start(
                    out=movs[0][32 * s4:32 * s4 + KDIM, 0:2 * NT],
                    in_=mv[0, s4, :, 0:2 * NT])
            for s4 in range(NSEG):
                nc.gpsimd.dma_start(
                    out=movs[0][32 * s4:32 * s4 + KDIM, 2 * NT:SEGW],
                    in_=mv[0, s4, :, 2 * NT:SEGW])
            for pair in range(1, PAIRS):
                for s4 in range(NSEG):
                    nc.gpsimd.dma_start(
                        out=movs[pair][32 * s4:32 * s4 + KDIM, :],
                        in_=mv[pair, s4])

            # t-chunks 0-1 / 2-3 / 4-6 each go to their own stage tile
            # (uniform sizes per pool) and drain as one DMA per group as
            # soon as the group's evacs complete. The very last group
            # (pair 3) instead uses three per-t tiles so the kernel tail
            # is one small drain, not a 1.4 MB one.
            GROUPS = [[0, 1], [2, 3], [4, 5, 6]]
            GPOOLS = [sg0p, sg1p, sg2p]
            tidx = 0
            for pair in range(PAIRS):
                mov = movs[pair]
                for gi, grp in enumerate(GROUPS):
                    last_tail = pair == PAIRS - 1 and gi == 2
                    if last_tail:
                        subgroups = [[t] for t in grp]
                    else:
                        subgroups = [grp]
                    for sg in subgroups:
                        gw = NT * len(sg)
                        pool = stailp if last_tail else GPOOLS[gi]
                        st = pool.tile([128, NSEG * gw], f16, tag="st",
                                       name=f"st_{pair}_{sg[0]}")
                        for t in sg:
                            n0 = t * NT
                            pt = psump.tile([128, 4 * BANK], f32, tag="pt")
                            for seg in range(NSEG):
                                p0 = 32 * seg
                                nc.tensor.matmul(
                                    pt[:, BANK * seg:BANK * seg + NT],
                                    w2_t[p0:p0 + KDIM, :],
                                    mov[p0:p0 + KDIM, n0:n0 + NT],
                                    start=True, stop=True,
                                    tile_position=(p0, 0))
                            # Grouped PSUM->SBUF evacuation of all 4 banks
                            # with fused bias add, alternating Scalar/Vector.
                            in_ap = apx(pt[:], 0,
                                        [[4 * BANK, 128], [BANK, NSEG],
                                         [1, NT]])
                            out_ap = apx(st[:], NT * (t - sg[0]),
                                         [[NSEG * gw, 128], [gw, NSEG],
                                          [1, NT]])
                            if tidx % 2 == 0:
                                nc.scalar.activation(
                                    out_ap, in_ap,
                                    mybir.ActivationFunctionType.Identity,
                                    bias=b2_t[:, :])
                            else:
                                nc.vector.tensor_scalar_add(out_ap, in_ap,
                                                            b2_t[:, :])
                            tidx += 1
                        # Drain: stage cols gw*seg+j -> out cols
                        # SEGW*seg + NT*sg[0] + j.
                        nc.sync.dma_start(
                            out=apx(out[:], pair * 128 * NPIX + NT * sg[0],
                                    [[NPIX, 128], [SEGW, NSEG], [1, gw]]),
                            in_=st[:])
    nc.compile()
    return nc


def _get_nc():
    if "nc" not in _CACHE:
        _CACHE["nc"] = _build_bass()
    return _CACHE["nc"]


def _prep_inputs(x_padded, weight, bias):
    x = np.asarray(x_padded, dtype=np.float32)
    wt = np.asarray(weight, dtype=np.float32)
    bs = np.asarray(bias, dtype=np.float32)

    xs3 = x[:, -1, :, :]                              # [64, 114, 114]
    win = np.lib.stride_tricks.sliding_window_view(xs3, (KS, KS), axis=(1, 2))
    # [64, 112, 112, 3, 3] -> [64, 9, 12544] with row k = (i, j) shift
    mov_all = win.transpose(0, 3, 4, 1, 2).reshape(B, KS * KS, NPIX)
    # -> [cores, pairs, 2, 9, NSEG, SEGW] -> [cores, pairs, NSEG, 18, SEGW]
    mov_r = mov_all.reshape(NCORES, PAIRS, 2, KS * KS, NSEG, SEGW)
    mov_h = np.ascontiguousarray(
        mov_r.transpose(0, 1, 4, 2, 3, 5).reshape(
            NCORES, PAIRS, NSEG, KDIM, SEGW)).astype(np.float16)

    wl = np.ascontiguousarray(wt[:, -1, :, :]).reshape(COUT, KS * KS)
    w2 = np.zeros((128, 128), np.float16)
    for s in range(NSEG):
        w2[32 * s: 32 * s + 9, 0:64] = wl.T
        w2[32 * s + 9: 32 * s + 18, 64:128] = wl.T
    b2 = np.tile(bs, 2).reshape(128, 1).astype(np.float32)
    return mov_h, w2, b2


def kernel(x_padded, weight, bias, in_height=112, in_width=112, **_unused):
    from concourse.bass_utils import run_bass_kernel_spmd

    mov_h, w2, b2 = _prep_inputs(x_padded, weight, bias)
    nc = _get_nc()
    in_maps = [
        {"mv": mov_h[c], "w2": w2, "b2": b2}
        for c in range(NCORES)
    ]
    res = run_bass_kernel_spmd(nc, in_maps, core_ids=list(range(NCORES)))
    outs = [
        np.asarray(res.results[c]["out"]).astype(np.float32).reshape(
            BL, COUT, H, W)
        for c in range(NCORES)
    ]
    return np.concatenate(outs, axis=0)


# revision 10
# speedup vs baseline: 1.1333x; 1.0593x over previous
"""Trainium2 Bass kernel for nn_CustomConv2D (degenerate conv: only the last
input channel contributes; 3x3 VALID conv -> 64 out channels + bias).

Strategy (v4):
  - Tolerance is rel_err < 2e-2; fp16 end-to-end gives ~2e-4, so the im2col
    moving data AND the output are fp16 (in 1.81 MB, out 12.85 MB per core;
    host upcasts the result).
  - Host: slice x_padded[:, -1], build the 9-row im2col per batch WITHOUT
    zero padding rows ([pairs, seg, 18, 3136]), shard batch across 8 cores.
  - Input: 16 DMAs (one per pair x segment) on the GpSimd/SWDGE queue so
    the Scalar sequencer stays free for evacuations; rows land directly at
    partition offsets 32s..32s+17.
  - Inner loop: for each t (7 x 448 pixels per segment) the 4 concurrent
    quadrant matmuls (stationary replicated at partition 0/32/64/96,
    block-diagonal over the batch pair) write 4 DISTINCT BANKS of one
    [128, 2048] PSUM tile. A single strided-AP op evacuates all four banks
    with fused bias add, alternating Scalar/Vector engines (7 grouped
    evacs per pair instead of 28 small ones; per-op overhead ~400 ns).
  - Evacs t=0..3 write stageA, t=4..6 write stageB (seg-major layout), so
    each stage drains as ONE large contiguous DMA (1.84 / 1.38 MB) with no
    AP overlap with later evacs -> only 8 output DMAs per core, all big.
"""

import sys

if "/opt/trn_rl_repo" not in sys.path:
    sys.path.insert(0, "/opt/trn_rl_repo")

import numpy as np

B, CIN, COUT, KS = 64, 64, 64, 3
H, W, HP, WP = 112, 112, 114, 114
NPIX = H * W          # 12544
IMG = HP * WP         # 12996
NCORES = 8
BL = B // NCORES      # 8 local batches per core
PAIRS = BL // 2       # 4
KDIM = 2 * KS * KS    # 18
NSEG = 4              # pixel segments per pair (partition offsets 0/32/64/96)
SEGW = NPIX // NSEG   # 3136
NT = 448              # pixels per matmul; 7 * 448 == 3136
TPS = SEGW // NT      # 7 matmul tiles per segment
BANK = 512            # f32 elems per PSUM bank
TA = 4                # t-chunks staged in stageA (drained early)
TB = TPS - TA         # 3 t-chunks in stageB
WA, WB = TA * NT, TB * NT   # 1792, 1344

_CACHE = {}


def _build_bass():
    import bass_rust
    import concourse.bass as bass
    import concourse.bacc as bacc
    import concourse.mybir as mybir
    from concourse.tile import TileContext

    f32 = mybir.dt.float32
    f16 = mybir.dt.float16

    def apx(base_ap, extra_off, dims):
        """Custom access pattern on a tile: dims = [[stride, size], ...]
        (elements), first dim must be the partition dim."""
        return bass_rust.AP(base_ap.tensor, base_ap.offset + extra_off, dims)

    nc = bacc.Bacc("TRN2", target_bir_lowering=False, debug=False)
    mv = nc.declare_dram_parameter("mv", [PAIRS, NSEG, KDIM, SEGW], f16,
                                   isOutput=False)
    w2 = nc.declare_dram_parameter("w2", [128, 128], f16, isOutput=False)
    b2 = nc.declare_dram_parameter("b2", [128, 1], f32, isOutput=False)
    out = nc.declare_dram_parameter("out", [BL * COUT, NPIX], f16,
                                    isOutput=True)

    with TileContext(nc) as tc:
        with (
            tc.tile_pool(name="consts", bufs=1) as consts,
            tc.tile_pool(name="movp", bufs=PAIRS) as movp,
            tc.tile_pool(name="p3584", bufs=3) as p3584,
            tc.tile_pool(name="p5376", bufs=2) as p5376,
            tc.tile_pool(name="p7168", bufs=2) as p7168,
            tc.tile_pool(name="p1792", bufs=1) as p1792,
            tc.tile_pool(name="psump", bufs=2, space="PSUM") as psump,
        ):
            w2_t = consts.tile([128, 128], f16)
            nc.scalar.dma_start(out=w2_t[:], in_=w2[:])
            b2_t = consts.tile([128, 1], f32)
            nc.sync.dma_start(out=b2_t[:], in_=b2[:])

            # Prefetch every pair's moving tile, ALL on the single SWDGE
            # queue (idle Pool sequencer): one queue = FIFO order, so pair
            # 0's tiles complete first. Pair 0 gets 4 small "starter"
            # chunks (t=0/1 columns) AHEAD of everything so its first
            # matmuls begin ASAP.
            movs = [movp.tile([128, SEGW], f16, tag="mov", name=f"mov_{p}")
                    for p in range(PAIRS)]
            for s4 in range(NSEG):
                nc.gpsimd.dma_start(
                    out=movs[0][32 * s4:32 * s4 + KDIM, 0:2 * NT],
                    in_=mv[0, s4, :, 0:2 * NT])
            for s4 in range(NSEG):
                nc.gpsimd.dma_start(
                    out=movs[0][32 * s4:32 * s4 + KDIM, 2 * NT:SEGW],
                    in_=mv[0, s4, :, 2 * NT:SEGW])
            for pair in range(1, PAIRS):
                for s4 in range(NSEG):
                    nc.gpsimd.dma_start(
                        out=movs[pair][32 * s4:32 * s4 + KDIM, :],
                        in_=mv[pair, s4])

            # Stage-group layout per pair. Drain descriptor chunk size is
            # group_width*2 bytes, so bulk groups stay >= 4 t's wide
            # (3584 B chunks, full DMA rate). Pair 0 uses thin 2-t groups
            # only to prime the output stream sooner; pair 3's tail group
            # [6] drains per seg-half so the kernel's last drain is small
            # AND still has 1792 B chunks.
            GROUPS = {
                0: [[0, 1], [2, 3], [4, 5, 6]],
                1: [[0, 1, 2, 3], [4, 5, 6]],
                2: [[0, 1, 2, 3], [4, 5, 6]],
                3: [[0, 1, 2, 3], [4, 5], [6]],
            }
            POOL = {3584: p3584, 5376: p5376, 7168: p7168, 1792: p1792}
            tidx = 0
            for pair in range(PAIRS):
                mov = movs[pair]
                for grp in GROUPS[pair]:
                    gw = NT * len(grp)
                    st = POOL[NSEG * gw].tile([128, NSEG * gw], f16,
                                              tag=f"st{gw}",
                                              name=f"st_{pair}_{grp[0]}")
                    for t in grp:
                        n0 = t * NT
                        pt = psump.tile([128, 4 * BANK], f32, tag="pt")
                        for seg in range(NSEG):
                            p0 = 32 * seg
                            nc.tensor.matmul(
                                pt[:, BANK * seg:BANK * seg + NT],
                                w2_t[p0:p0 + KDIM, :],
                                mov[p0:p0 + KDIM, n0:n0 + NT],
                                start=True, stop=True,
                                tile_position=(p0, 0))
                        # Grouped PSUM->SBUF evacuation of all 4 banks
                        # with fused bias add, alternating Scalar/Vector.
                        in_ap = apx(pt[:], 0,
                                    [[4 * BANK, 128], [BANK, NSEG],
                                     [1, NT]])
                        out_ap = apx(st[:], NT * (t - grp[0]),
                                     [[NSEG * gw, 128], [gw, NSEG],
                                      [1, NT]])
                        if tidx % 2 == 0:
                            nc.scalar.activation(
                                out_ap, in_ap,
                                mybir.ActivationFunctionType.Identity,
                                bias=b2_t[:, :])
                        else:
                            nc.vector.tensor_scalar_add(out_ap, in_ap,
                                                        b2_t[:, :])
                        tidx += 1
                    # Drain: stage cols gw*seg+j -> out cols
                    # SEGW*seg + NT*grp[0] + j. The final tail group goes
                    # out as two seg-half DMAs so the very last transfer
                    # is only ~0.23 MB.
                    if pair == PAIRS - 1 and grp == [6]:
                        for sh in range(2):
                            nc.sync.dma_start(
                                out=apx(out[:],
                                        pair * 128 * NPIX + NT * grp[0]
                                        + 2 * sh * SEGW,
                                        [[NPIX, 128], [SEGW, 2], [1, gw]]),
                                in_=st[:, 2 * sh * gw:2 * (sh + 1) * gw])
                    else:
                        nc.sync.dma_start(
                            out=apx(out[:], pair * 128 * NPIX + NT * grp[0],
                                    [[NPIX, 128], [SEGW, NSEG], [1, gw]]),
                            in_=st[:])
    nc.compile()
    return nc


def _get_nc():
    if "nc" not in _CACHE:
        _CACHE["nc"] = _build_bass()
    return _CACHE["nc"]


def _prep_inputs(x_padded, weight, bias):
    x = np.asarray(x_padded, dtype=np.float32)
    wt = np.asarray(weight, dtype=np.float32)
    bs = np.asarray(bias, dtype=np.float32)

    xs3 = x[:, -1, :, :]                              # [64, 114, 114]
    win = np.lib.stride_tricks.sliding_window_view(xs3, (KS, KS), axis=(1, 2))
    # [64, 112, 112, 3, 3] -> [64, 9, 12544] with row k = (i, j) shift
    mov_all = win.transpose(0, 3, 4, 1, 2).reshape(B, KS * KS, NPIX)
    # -> [cores, pairs, 2, 9, NSEG, SEGW] -> [cores, pairs, NSEG, 18, SEGW]
    mov_r = mov_all.reshape(NCORES, PAIRS, 2, KS * KS, NSEG, SEGW)
    mov_h = np.ascontiguousarray(
        mov_r.transpose(0, 1, 4, 2, 3, 5).reshape(
            NCORES, PAIRS, NSEG, KDIM, SEGW)).astype(np.float16)

    wl = np.ascontiguousarray(wt[:, -1, :, :]).reshape(COUT, KS * KS)
    w2 = np.zeros((128, 128), np.float16)
    for s in range(NSEG):
        w2[32 * s: 32 * s + 9, 0:64] = wl.T
        w2[32 * s + 9: 32 * s + 18, 64:128] = wl.T
    b2 = np.tile(bs, 2).reshape(128, 1).astype(np.float32)
    return mov_h, w2, b2


def kernel(x_padded, weight, bias, in_height=112, in_width=112, **_unused):
    from concourse.bass_utils import run_bass_kernel_spmd

    mov_h, w2, b2 = _prep_inputs(x_padded, weight, bias)
    nc = _get_nc()
    in_maps = [
        {"mv": mov_h[c], "w2": w2, "b2": b2}
        for c in range(NCORES)
    ]
    res = run_bass_kernel_spmd(nc, in_maps, core_ids=list(range(NCORES)))
    outs = [
        np.asarray(res.results[c]["out"]).astype(np.float32).reshape(
            BL, COUT, H, W)
        for c in range(NCORES)
    ]
    return np.concatenate(outs, axis=0)
